# revision 1
# baseline (speedup 1.0000x reference)
"""Trainium2 Bass kernel for nn_FIoUCriterion (pairwise-overlap IoU-style loss).

Strategy (8 NeuronCores, data-parallel over batch), variant "ct" (col-tiled):
  - Host: y = max(x,-1)+1 (= 2*m), cast fp8 e3m4 (rel-err on final loss ~8e-5),
    pre-transpose per core to pixel-major layout [128, pair, chunk, 130] where
    each chunk-block is [b_even rows (64) | ones | b_odd rows (64) | ones].
    Plain contiguous DMA (4.2 MB/core) replaces v2's 8 MB xbar-transpose DMA.
  - Device (per core): per 128-pixel chunk and per 2-batch pair, TWO M=64
    matmuls into different PE column-groups (tile_position auto (0,0)/(0,64)):
      top: out[0:64,  0:65] = b_even.T @ [b_even | 1]   -> [gram | s]
      bot: out[64:128,0:65] = b_odd.T  @ [b_odd | 1]    -> [gram | s]
    The ones columns are interleaved so both rhs slices are contiguous and
    both halves put s at out col 64 (one reciprocal + one scale op).  No
    separate s-matmuls and no on-device relu.  Pair-major streaming lets
    pair0's epilogue overlap pair1's matmul stream.
  - Epilogue per pair: C = gram/s per half, PE transposes per 64-block,
    cr_contrib = max(C, C^T) (valid since gram>=0), fold lower block via
    PE transpose (symmetric), AllGather + local-sum of the (64,64) partials
    across 8 cores, then loss = sum(|beta - cr_sum/64| * wgt2) with
    wgt2 = (wgt + wgt^T) / (2*sum(wgt)).
  - Scale bookkeeping: y = 2m  => gram_psum = 4*gram, s_psum = 2*s,
    C = 2*gram/s; sum over 32 batches then *1/64 gives mean cr.
"""

import numpy as np
import ml_dtypes

N_CORES = 8
B, N, H, W = 32, 64, 128, 128
HW = H * W
B_LOC = B // N_CORES          # 4 batches per core
ROWS = B_LOC * N              # 256
N_PAIRS = B_LOC // 2          # 2 stacked pairs per core
N_CHUNK = HW // 128           # 128 pixel chunks
CPC = 130                     # cols per chunk-block (64 + ones + 64 + ones)
N_SEPARATE = 7
N_FLEXIBLE = 2

_cached = {}


def _build_bass(with_collective: bool = True, bench_loop: int | None = None,
                phase: str = "full",
                calls_by_pair=None, dual_ring: bool = False):
    import contextlib
    import concourse.bacc as bacc
    import concourse.mybir as mybir
    import concourse.tile as tile

    f32 = mybir.dt.float32
    f8 = mybir.dt.float8e3
    Alu = mybir.AluOpType

    nc = bacc.Bacc("TRN2", target_bir_lowering=False, debug=False, num_devices=N_CORES)
    xt = nc.dram_tensor("xt", [128, N_PAIRS * N_CHUNK * CPC], f8, kind="ExternalInput")
    beta_d = nc.dram_tensor("beta", [N, N], f32, kind="ExternalInput")
    wgt2_d = nc.dram_tensor("wgt2", [N, N], f32, kind="ExternalInput")
    loss_d = nc.dram_tensor("loss", [1, 1], f32, kind="ExternalOutput")

    def emit(tc, const, stream, ep, gpsum, tpsum, dram):
        # --- constants ---
        ones_f32 = const.tile([N, 1], f32)
        nc.vector.memset(ones_f32[:], 1.0)
        ident = const.tile([128, 128], f32)
        from concourse import masks as masks_lib
        masks_lib.make_identity(nc, ident[:])
        beta_t = const.tile([N, N], f32)
        nc.sync.dma_start(beta_t[:], beta_d[:])
        wgt2_t = const.tile([N, N], f32)
        nc.sync.dma_start(wgt2_t[:], wgt2_d[:])

        # --- PSUM accumulators: [gram | s] per pair; top half = b_even,
        # bottom half = b_odd (both with s at col 64) ---
        g_acc = [gpsum.tile([128, 65], f32, name=f"g_acc{p}") for p in range(N_PAIRS)]

        bench_cm = (tc.For_i(0, bench_loop, 1, hint_engines=(mybir.EngineType.PE,))
                    if bench_loop else contextlib.nullcontext())
        bench_cm.__enter__()

        f0 = ep.tile([N, N], f32)        # pair0's folded (64,64) partial
        a1 = ep.tile([N, N], f32)
        crl = ep.tile([N, N], f32)

        def chunk_matmuls(p, t, k, first, last):
            base = k * CPC
            nc.tensor.matmul(g_acc[p][0:64, 0:65],
                             lhsT=t[:, base:base + 64],
                             rhs=t[:, base:base + 65],
                             start=first, stop=last)
            nc.tensor.matmul(g_acc[p][64:128, 0:65],
                             lhsT=t[:, base + 65:base + 129],
                             rhs=t[:, base + 65:base + 130],
                             start=first, stop=last)

        def pair_epilogue(p):
            # C = gram / s  (s at col 64 for both halves)
            r = ep.tile([128, 1], f32, name=f"r{p}")
            nc.vector.reciprocal(r[:], g_acc[p][:, 64:65])
            C = ep.tile([128, 64], f32, name=f"C{p}")
            nc.vector.tensor_scalar_mul(C[:], g_acc[p][:, 0:64], r[:])
            # one full-width transpose: CT2[:, 0:64] = C_top^T, [:, 64:128] =
            # C_bot^T -- all on partitions 0:64 (transpose out must be at
            # PSUM partition 0)
            CT2 = tpsum.tile([64, 128], f32, name=f"CT{p}", tag="CT", bufs=1)
            nc.tensor.transpose(CT2[:], C[:], ident[:])
            mxt = ep.tile([N, N], f32, name=f"mxt{p}")
            nc.vector.tensor_max(mxt[:], C[0:64, :], CT2[:, 0:64])
            # bottom block: copy C_bot^T to SBUF, re-transpose for the other
            # orientation, then max -- already folded to partitions 0:64
            cpB = ep.tile([N, N], f32, name=f"cpB{p}")
            nc.vector.tensor_copy(cpB[:], CT2[:, 64:128])
            TB = tpsum.tile([N, N], f32, name=f"TB{p}", tag="TB", bufs=1)
            nc.tensor.transpose(TB[:], cpB[:], ident[0:64, 0:64])
            mxb = ep.tile([N, N], f32, name=f"mxb{p}")
            nc.vector.tensor_max(mxb[:], cpB[:], TB[:])
            if p == 0:
                # runs while pair1 is still streaming -- free
                nc.vector.tensor_add(f0[:], mxt[:], mxb[:])
            else:
                nc.vector.tensor_add(a1[:], f0[:], mxt[:])
                nc.vector.tensor_add(crl[:], a1[:], mxb[:])

        if phase == "pe":
            # pure PE-rate probe: one resident tile, full matmul count
            Xc = 16
            t = stream.tile([128, Xc * CPC], f8, name="tpe", tag="tpe", bufs=1)
            nc.sync.dma_start(t[:], xt[:, 0:Xc * CPC])
            for rep in range(N_CHUNK // Xc):
                for k in range(Xc):
                    first = (rep == 0 and k == 0)
                    last = (rep == N_CHUNK // Xc - 1 and k == Xc - 1)
                    for p in range(N_PAIRS):
                        chunk_matmuls(p, t, k, first, last)
            lout0 = ep.tile([1, 1], f32)
            nc.vector.memset(lout0[:], 0.0)
            nc.sync.dma_start(loss_d[:], lout0[:])
            bench_cm.__exit__(None, None, None)
            return

        # --- streaming: plain contiguous DMA, pair-major; pair0's epilogue
        # overlaps pair1's matmul stream ---
        CALLS_BY_PAIR = calls_by_pair or [
            [8, 24, 48, 48],          # pair0: small first call fills the pipe fast
            [48, 48, 32],             # pair1
        ]
        for p in range(N_PAIRS if phase != "noop" else 0):
            CALLS = CALLS_BY_PAIR[p]
            assert sum(CALLS) == N_CHUNK
            c0 = 0
            for ci, Xc in enumerate(CALLS):
                t = stream.tile([128, Xc * CPC], f8, name="t",
                                tag=f"t{p}_{ci}", bufs=1)
                eng = nc.sync if (p == 0 or not dual_ring) else nc.scalar
                eng.dma_start(
                    t[:], xt[:, (p * N_CHUNK + c0) * CPC:(p * N_CHUNK + c0 + Xc) * CPC])
                if phase != "dma":
                    for k in range(Xc):
                        first = (ci == 0 and k == 0)
                        last = (ci == len(CALLS) - 1 and k == Xc - 1)
                        chunk_matmuls(p, t, k, first, last)
                c0 += Xc
            if phase == "full":
                pair_epilogue(p)

        if phase in ("noop", "dma", "stream"):
            lout0 = ep.tile([1, 1], f32)
            nc.vector.memset(lout0[:], 0.0)
            nc.sync.dma_start(loss_d[:], lout0[:])
            bench_cm.__exit__(None, None, None)
            return

        # --- combine partials across the 8 cores ---
        # AllGather (floor ~4.6us on 8 cores) + local sum beats AllReduce
        # (floor ~9.7us) at this size.
        if with_collective:
            cc_in = dram.tile([N, N], f32)
            cc_ag = dram.tile([N_CORES * N, N], f32, addr_space="Shared")
            nc.sync.dma_start(cc_in[:], crl[:])
            nc.gpsimd.collective_compute(
                "AllGather", Alu.bypass,
                replica_groups=[list(range(N_CORES))],
                ins=[cc_in.opt()], outs=[cc_ag.opt()],
            )
            # gather back as (64, r, 64): S[i, r, j] = AG[r*64+i, j]
            sg = ep.tile([N, N_CORES * N], f32)
            nc.sync.dma_start(
                sg[:].rearrange("i (r j) -> i r j", r=N_CORES),
                cc_ag[:].rearrange("(r i) j -> i r j", r=N_CORES))
            crs = ep.tile([N, N], f32)
            # reduce over r: view free dim as (j outer, r inner) and reduce X
            nc.vector.tensor_reduce(
                crs[:], sg[:].rearrange("i (r j) -> i j r", r=N_CORES),
                mybir.AxisListType.X, Alu.add)
        else:
            crs = crl

        # --- final reduction ---
        u = ep.tile([N, N], f32)
        # u = (crs * 1/64) - beta
        nc.vector.scalar_tensor_tensor(u[:], crs[:], 1.0 / 64.0, beta_t[:],
                                       Alu.mult, Alu.subtract)
        v = ep.tile([N, N], f32)
        nc.vector.tensor_mul(v[:], u[:], wgt2_t[:])
        vr = ep.tile([N, 1], f32)
        nc.vector.tensor_reduce(vr[:], v[:], mybir.AxisListType.X, Alu.add,
                                apply_absolute_value=True)
        lps = tpsum.tile([1, 1], f32)
        nc.tensor.matmul(lps[:], lhsT=vr[:], rhs=ones_f32[:], start=True, stop=True)
        lout = ep.tile([1, 1], f32)
        nc.vector.tensor_copy(lout[:], lps[:])
        nc.sync.dma_start(loss_d[:], lout[:])

        bench_cm.__exit__(None, None, None)

    with tile.TileContext(nc) as tc:
        with tc.tile_pool(name="const", bufs=1) as const, \
             tc.tile_pool(name="stream", bufs=1) as stream, \
             tc.tile_pool(name="ep", bufs=1) as ep, \
             tc.tile_pool(name="gpsum", bufs=1, space="PSUM") as gpsum, \
             tc.tile_pool(name="tpsum", bufs=1, space="PSUM") as tpsum, \
             tc.tile_pool(name="dram", bufs=1, space="DRAM") as dram:
            emit(tc, const, stream, ep, gpsum, tpsum, dram)

    nc.compile()
    return nc


def _host_prep(masks: np.ndarray, nodes: np.ndarray):
    """Returns per-core input dicts (xt/beta/wgt2) for the 8 cores."""
    y = np.maximum(masks.astype(np.float32), -1.0) + 1.0      # (32,64,128,128) = 2m
    y8 = y.reshape(B, N, HW).astype(ml_dtypes.float8_e3m4)

    t = np.where(nodes < N_SEPARATE, 0, np.where(nodes < N_SEPARATE + N_FLEXIBLE, 1, 2))
    ti, tj = t[:, None], t[None, :]
    has_f = (ti == 1) | (tj == 1)
    has_a = (ti == 2) | (tj == 2)
    include = ~(has_f & ~has_a)
    beta = ((ti == 2) ^ (tj == 2)).astype(np.float32)
    triu = np.triu(np.ones((N, N), bool), k=1)
    wgt = (include & triu).astype(np.float64)
    wgt2 = ((wgt + wgt.T) / (2.0 * wgt.sum())).astype(np.float32)

    in_maps = []
    for c in range(N_CORES):
        yc = y8[c * B_LOC:(c + 1) * B_LOC].reshape(ROWS, N_CHUNK, 128)
        yt = yc.transpose(2, 1, 0)                  # [pixel, chunk, row] (128,128,256)
        xt = np.empty((128, N_PAIRS, N_CHUNK, CPC), dtype=ml_dtypes.float8_e3m4)
        for p in range(N_PAIRS):
            # [b_even rows (64) | ones | b_odd rows (64) | ones]
            xt[:, p, :, 0:64] = yt[:, :, p * 128:p * 128 + 64]
            xt[:, p, :, 65:129] = yt[:, :, p * 128 + 64:(p + 1) * 128]
        xt[:, :, :, 64] = np.float32(1.0)
        xt[:, :, :, 129] = np.float32(1.0)
        in_maps.append({
            "xt": np.ascontiguousarray(xt.reshape(128, N_PAIRS * N_CHUNK * CPC)),
            "beta": beta, "wgt2": wgt2,
        })
    return in_maps


def kernel(masks: np.ndarray, nodes: np.ndarray) -> np.ndarray:
    from concourse.bass_utils import run_bass_kernel_spmd

    masks = np.asarray(masks, dtype=np.float32)
    nodes = np.asarray(nodes)
    in_maps = _host_prep(masks, nodes)

    if "nc" not in _cached:
        _cached["nc"] = _build_bass()
    nc = _cached["nc"]

    try:
        res = run_bass_kernel_spmd(nc, in_maps, core_ids=list(range(N_CORES)))
    except Exception:
        res = run_bass_kernel_spmd(nc, in_maps, core_ids=list(range(N_CORES)))
    loss = np.float32(res.results[0]["loss"][0, 0])
    return np.asarray(loss, dtype=np.float32).reshape(())

